# revision 1
# baseline (speedup 1.0000x reference)
"""Trainium2 Bass kernel for the PINN-style loss problem.

Math: a 6-layer tanh MLP u(x,t) (2->50x5->1) is evaluated with forward-mode
jets (u, u_x, u_t, u_xxx) at N=10000 points. The per-param loss
  loss_p = mean_n (u_t + a_p*u*u_x + b_p*u_xxx + c_p*u_x)^2
collapses to a quadratic form in the 4x4 Gram matrix of
g_n = [u*u_x, u_xxx, u_x, u_t]:  loss_p = (s0 + 2 p.s1 + p^T S p)/N.

Sharding: x is split into 8 slices of 1250 points (one per NeuronCore);
each core builds its partial Gram, an AllReduce sums them, then each core
evaluates the quadratic form for its 625-row slice of para.

Device layout: points are packed 2-per-partition-block (block-diagonal
weights, K=100), free dim 640 per block (block0: 640 real points,
block1: 610 real + 30 zero-padded, masked out before the Gram matmul).
"""

import os
import sys
import numpy as np

for _p in ("/opt/trn_rl_repo",):
    if os.path.isdir(_p) and _p not in sys.path:
        sys.path.append(_p)

import concourse.bass as bass
import concourse.bacc as bacc
import concourse.mybir as mybir
import concourse.tile as tile
from concourse import bass_utils

F32 = mybir.dt.float32
F32R = mybir.dt.float32r
AF = mybir.ActivationFunctionType
ALU = mybir.AluOpType

NCORES = 8
NPTS = 10000
NPC = NPTS // NCORES       # 1250 points per core
PPC = 5000 // NCORES       # 625 para rows per core
FD = 640                   # free dim per block (block0 full, block1 padded)
B1 = NPC - FD              # 610 real points in block1
HB = 100                   # 2 blocks x 50 hidden units
CHUNKS = ((0, 512), (512, 128))      # matmul free-dim chunks (psum bank limit)
PCH = ((0, 512), (512, PPC - 512))   # para free-dim chunks

SDT = F32R                 # tower stream/weight dtype (f32r: 1 cyc/row matmul)
WARM_CC = True             # early dummy collective to warm the CC path


def _mm(nc, out, lhsT, rhs, start=True, stop=True):
    nc.tensor.matmul(out, lhsT, rhs, start=start, stop=stop)


def _mm_chunks(nc, out_tile, lhsT, rhs_tile, chunks=CHUNKS):
    for off, w in chunks:
        _mm(nc, out_tile[:, off:off + w], lhsT, rhs_tile[:, off:off + w])


def build_program(stage="full"):
    nc = bacc.Bacc("TRN2", target_bir_lowering=False, debug=False)

    h0_d = nc.dram_tensor("h0", [4, FD], SDT, kind="ExternalInput")
    paraT_d = nc.dram_tensor("paraT", [3, PPC], F32, kind="ExternalInput")
    w1t_d = nc.dram_tensor("w1t", [4, HB], SDT, kind="ExternalInput")
    wb_d = nc.dram_tensor("wb", [HB, 400], SDT, kind="ExternalInput")
    w6p_d = nc.dram_tensor("w6p", [HB, 2], SDT, kind="ExternalInput")
    vecs_d = nc.dram_tensor("vecs", [HB, 10], F32, kind="ExternalInput")
    b6bc_d = nc.dram_tensor("b6bc", [128, 2], F32, kind="ExternalInput")
    if stage == "full":
        loss_d = nc.dram_tensor("loss", [1, PPC], F32, kind="ExternalOutput")
    elif stage == "tower":
        loss_d = nc.dram_tensor("dbg", [HB, FD], F32, kind="ExternalOutput")
    elif stage == "para2":
        loss_d = nc.dram_tensor("loss", [1, PPC], F32, kind="ExternalOutput")
    else:  # l6 / cc / para1
        loss_d = nc.dram_tensor("dbg", [5, 4], F32, kind="ExternalOutput")

    with tile.TileContext(nc) as tc:
        _body(tc, nc, h0_d, paraT_d, w1t_d, wb_d, w6p_d, vecs_d, b6bc_d, loss_d,
              stage=stage)
    nc.compile()
    return nc


def _body(tc, nc, h0_d, paraT_d, w1t_d, wb_d, w6p_d, vecs_d, b6bc_d, loss_d,
          stage="full"):
    import contextlib

    ctx = contextlib.ExitStack()
    with ctx:
        cpool = ctx.enter_context(tc.tile_pool(name="const", bufs=1))
        spool = ctx.enter_context(tc.tile_pool(name="streams", bufs=2))
        tpool = ctx.enter_context(tc.tile_pool(name="trans", bufs=2))
        dpool = ctx.enter_context(tc.tile_pool(name="dram", bufs=1, space="DRAM"))

        # ---- load constants ----
        h0 = cpool.tile([4, FD], SDT, tag="h0")
        paraT = cpool.tile([3, PPC], F32, tag="paraT")
        w1t = cpool.tile([4, HB], SDT, tag="w1t")
        wb = cpool.tile([HB, 400], SDT, tag="wb")
        w6p = cpool.tile([HB, 2], SDT, tag="w6p")
        vecs = cpool.tile([HB, 10], F32, tag="vecs")
        b6bc = cpool.tile([128, 2], F32, tag="b6bc")
        ones3 = cpool.tile([3, 1], F32, tag="ones3")
        for t, d in ((h0, h0_d), (paraT, paraT_d), (w1t, w1t_d), (wb, wb_d),
                     (w6p, w6p_d), (vecs, vecs_d), (b6bc, b6bc_d)):
            nc.sync.dma_start(t[:], d[:])
        nc.vector.memset(ones3[:], 1.0)

        if WARM_CC:
            win = dpool.tile([1, 1], F32, tag="win")
            wout = dpool.tile([1, 1], F32, tag="wout")
            nc.gpsimd.dma_start(win[:], ones3[0:1, 0:1])
            nc.gpsimd.collective_compute(
                "AllReduce", ALU.add,
                replica_groups=[list(range(NCORES))],
                ins=[win.opt()], outs=[wout.opt()],
            )

        cx = vecs[:, 0:1]
        ct = vecs[:, 1:2]
        cx2 = vecs[:, 2:3]
        cx3 = vecs[:, 3:4]

        def bb(layer):  # bias vector for layer 1..5
            return vecs[:, 3 + layer:4 + layer]

        neg2 = vecs[:, 9:10]

        v = nc.vector
        s = nc.scalar
        g = nc.gpsimd

        a5 = ax5 = at5 = axxx5 = None

        with tc.tile_pool(name="ztw", bufs=3, space="PSUM") as zpool:
            # ---------- layer 1 ----------
            z = zpool.tile([HB, FD], F32, tag="ztw")
            _mm_chunks(nc, z, w1t[:], h0)
            a = spool.tile([HB, FD], SDT, tag="a")
            s.activation(a[:], z[:], AF.Tanh, bias=bb(1))
            asq = tpool.tile([HB, FD], F32, tag="asq")
            s.activation(asq[:], a[:], AF.Square)
            f1 = tpool.tile([HB, FD], F32, tag="f1")
            s.activation(f1[:], asq[:], AF.Identity, scale=-1.0, bias=1.0)
            h6 = tpool.tile([HB, FD], F32, tag="h6")
            s.activation(h6[:], asq[:], AF.Identity, scale=6.0, bias=neg2)
            ax = spool.tile([HB, FD], SDT, tag="ax")
            v.tensor_scalar(ax[:], f1[:], cx, None, ALU.mult)
            at = spool.tile([HB, FD], SDT, tag="at")
            v.tensor_scalar(at[:], f1[:], ct, None, ALU.mult)
            af1 = tpool.tile([HB, FD], F32, tag="p1")
            v.tensor_tensor(af1[:], a[:], f1[:], ALU.mult)
            axx = spool.tile([HB, FD], SDT, tag="axx")
            v.tensor_scalar(axx[:], af1[:], cx2, -2.0, ALU.mult, ALU.mult)
            f3 = tpool.tile([HB, FD], F32, tag="p2")
            g.tensor_tensor(f3[:], f1[:], h6[:], ALU.mult)
            axxx = spool.tile([HB, FD], SDT, tag="axxx")
            v.tensor_scalar(axxx[:], f3[:], cx3, None, ALU.mult)

            # ---------- layers 2..5 ----------
            for layer in range(2, 6):
                W = wb[:, 100 * (layer - 2):100 * (layer - 1)]
                last = layer == 5

                z = zpool.tile([HB, FD], F32, tag="ztw")
                _mm_chunks(nc, z, W, a)
                a_n = spool.tile([HB, FD], SDT, tag="a")
                s.activation(a_n[:], z[:], AF.Tanh, bias=bb(layer))

                zt = zpool.tile([HB, FD], F32, tag="ztw")
                _mm_chunks(nc, zt, W, at)
                asq = tpool.tile([HB, FD], F32, tag="asq")
                s.activation(asq[:], a_n[:], AF.Square)
                f1 = tpool.tile([HB, FD], F32, tag="f1")
                s.activation(f1[:], asq[:], AF.Identity, scale=-1.0, bias=1.0)
                at_n = spool.tile([HB, FD], SDT, tag="at")
                v.tensor_tensor(at_n[:], f1[:], zt[:], ALU.mult)

                zx = zpool.tile([HB, FD], F32, tag="ztw")
                _mm_chunks(nc, zx, W, ax)
                h6 = tpool.tile([HB, FD], F32, tag="h6")
                s.activation(h6[:], asq[:], AF.Identity, scale=6.0, bias=neg2)
                ax_n = spool.tile([HB, FD], SDT, tag="ax")
                v.tensor_tensor(ax_n[:], f1[:], zx[:], ALU.mult)
                w2 = tpool.tile([HB, FD], F32, tag="w2")
                s.activation(w2[:], zx[:], AF.Square)
                P = tpool.tile([HB, FD], F32, tag="p1")
                v.tensor_tensor(P[:], a_n[:], zx[:], ALU.mult)
                zx3 = tpool.tile([HB, FD], F32, tag="zx3")
                v.tensor_tensor(zx3[:], w2[:], zx[:], ALU.mult)

                zxx = zpool.tile([HB, FD], F32, tag="ztw")
                _mm_chunks(nc, zxx, W, axx)
                if not last:
                    gt = tpool.tile([HB, FD], F32, tag="g")
                    g.tensor_tensor(gt[:], a_n[:], w2[:], ALU.mult)
                    inner = tpool.tile([HB, FD], F32, tag="inner")
                    v.scalar_tensor_tensor(inner[:], gt[:], -2.0, zxx[:],
                                           ALU.mult, ALU.add)
                m = tpool.tile([HB, FD], F32, tag="p2")
                v.tensor_tensor(m[:], P[:], zxx[:], ALU.mult)
                if not last:
                    axx_n = spool.tile([HB, FD], SDT, tag="axx")
                    g.tensor_tensor(axx_n[:], f1[:], inner[:], ALU.mult)

                zxxx = zpool.tile([HB, FD], F32, tag="ztw")
                _mm_chunks(nc, zxxx, W, axxx)
                i3a = tpool.tile([HB, FD], F32, tag="i3a")
                v.scalar_tensor_tensor(i3a[:], m[:], -6.0, zxxx[:],
                                       ALU.mult, ALU.add)
                n_t = tpool.tile([HB, FD], F32, tag="n")
                g.tensor_tensor(n_t[:], h6[:], zx3[:], ALU.mult)
                i3 = tpool.tile([HB, FD], F32, tag="i3")
                g.tensor_tensor(i3[:], i3a[:], n_t[:], ALU.add)
                axxx_n = spool.tile([HB, FD], SDT, tag="axxx")
                v.tensor_tensor(axxx_n[:], f1[:], i3[:], ALU.mult)

                a, at, ax, axxx = a_n, at_n, ax_n, axxx_n
                if not last:
                    axx = axx_n

            a5, ax5, at5, axxx5 = a, ax, at, axxx

        if stage == "tower":
            nc.sync.dma_start(loss_d[:], axxx5[:].bitcast(F32))
            return

        # ---------- layer 6 + Gram ----------
        # chunk tiles: [128 points, 10] cols: s-major pairs (b0,b1) for
        # s=0 uux, 1 uxxx, 2 ux, 3 ut; cols 8:10 = u.
        with tc.tile_pool(name="l6c", bufs=2, space="PSUM") as l6p, \
             tc.tile_pool(name="psmall", bufs=1, space="PSUM") as pps:
            G = pps.tile([4, 4], F32, tag="gram")
            Gr4 = pps.tile([1, 4], F32, tag="gram_r")
            for c in range(5):
                lo = 128 * c
                ch = l6p.tile([128, 10], F32, tag="l6c")
                _mm(nc, ch[:, 8:10], a5[:, lo:lo + 128], w6p[:])
                _mm(nc, ch[:, 2:4], axxx5[:, lo:lo + 128], w6p[:])
                _mm(nc, ch[:, 4:6], ax5[:, lo:lo + 128], w6p[:])
                _mm(nc, ch[:, 6:8], at5[:, lo:lo + 128], w6p[:])
                chS = tpool.tile([128, 10], F32, tag="l6s")
                v.tensor_copy(chS[:, 2:10], ch[:, 2:10])
                # uux = (u + b6) * ux
                v.scalar_tensor_tensor(chS[:, 0:2], chS[:, 8:10], b6bc[:128, 0:1],
                                       chS[:, 4:6], ALU.add, ALU.mult)
                chv = chS[:, 0:8].rearrange("p (s b) -> p b s", b=2, s=4)
                if c == 4 and B1 < FD:
                    # zero the padded block1 points before the Gram matmul
                    v.tensor_scalar(chv[:, 1, :], chv[:, 1, :], b6bc[:128, 1:2],
                                    None, ALU.mult)
                for b in range(2):
                    st = c == 0 and b == 0
                    sp = c == 4 and b == 1
                    nc.tensor.matmul(G[:], chv[:, b, :], chv[:, b, :],
                                     start=st, stop=sp)
                    # last Gram row (incl s0 = sum ut^2) at partition 0
                    nc.tensor.matmul(Gr4[:], chv[:, b, 3:4], chv[:, b, :],
                                     start=st, stop=sp)

            gS = cpool.tile([4, 4], F32, tag="gS")
            v.tensor_copy(gS[:], G[:])
            gS4 = cpool.tile([1, 4], F32, tag="gS4")
            v.tensor_copy(gS4[:], Gr4[:])

            if stage == "l6":
                nc.sync.dma_start(loss_d[0:4, :], gS[:])
                nc.sync.dma_start(loss_d[4:5, :], gS4[:])
                return

            # ---------- AllReduce the Gram (packed [5,4] bounce) ----------
            gin = dpool.tile([5, 4], F32, tag="gin")
            gout = dpool.tile([5, 4], F32, tag="gout")
            nc.gpsimd.dma_start(gin[0:4, :], gS[:])
            nc.gpsimd.dma_start(gin[4:5, :], gS4[:])
            nc.gpsimd.collective_compute(
                "AllReduce",
                ALU.add,
                replica_groups=[list(range(NCORES))],
                ins=[gin.opt()],
                outs=[gout.opt()],
            )
            Gr = cpool.tile([4, 4], F32, tag="Gr")
            nc.gpsimd.dma_start(Gr[:], gout[0:4, :])
            GrR = cpool.tile([1, 4], F32, tag="GrR")
            nc.gpsimd.dma_start(GrR[:], gout[4:5, :])

            if stage == "cc":
                nc.sync.dma_start(loss_d[0:4, :], Gr[:])
                nc.sync.dma_start(loss_d[4:5, :], GrR[:])
                return

            # ---------- para quadratic form ----------
            # loss = (s0 + 2 p.s1 + p^T S p) / N
            # S = Gr[0:3,0:3], s1 = Gr[0:3,3], s0 = GrR[0,3]
            s1d = cpool.tile([3, 1], F32, tag="s1d")
            s.activation(s1d[:], Gr[0:3, 3:4], AF.Copy, scale=2.0)
            PS = pps.tile([3, PPC], F32, tag="PS")
            for off, w in PCH:
                _mm(nc, PS[:, off:off + w], Gr[0:3, 0:3], paraT[:, off:off + w])
            H3 = cpool.tile([3, PPC], F32, tag="H3")
            v.scalar_tensor_tensor(H3[:], PS[:], s1d[:], paraT[:],
                                   ALU.add, ALU.mult)
            if stage == "para1":
                nc.sync.dma_start(loss_d[0:3, :], H3[:, 0:4])
                nc.sync.dma_start(loss_d[3:4, :], GrR[:])
                return
            LP = pps.tile([1, PPC], F32, tag="LP")
            for off, w in PCH:
                _mm(nc, LP[:, off:off + w], ones3[:], H3[:, off:off + w])
            lossS = cpool.tile([1, PPC], F32, tag="lossS")
            # loss = (LP + s0) / N  -- s0 folded via DVE scalar-AP add
            v.tensor_scalar(lossS[:], LP[:], GrR[0:1, 3:4], 1.0 / NPTS,
                            ALU.add, ALU.mult)
            nc.sync.dma_start(loss_d[:], lossS[:])


def prep_inputs(x, para, W1, b1, W2, b2, W3, b3, W4, b4, W5, b5, W6, b6):
    """Full inputs -> list of per-core input dicts (host-side shard/layout)."""
    f = np.float32
    x = np.asarray(x, f)
    para = np.asarray(para, f)
    Ws = [np.asarray(W, f) for W in (W1, W2, W3, W4, W5, W6)]
    bs = [np.asarray(b, f) for b in (b1, b2, b3, b4, b5, b6)]

    w1t = np.zeros((4, HB), f)
    w1t[0:2, 0:50] = Ws[0].T
    w1t[2:4, 50:100] = Ws[0].T
    wb = np.zeros((HB, 400), f)
    for i in range(4):
        W = Ws[i + 1]
        wb[0:50, 100 * i:100 * i + 50] = W.T
        wb[50:100, 100 * i + 50:100 * i + 100] = W.T
    w6p = np.zeros((HB, 2), f)
    w6p[0:50, 0] = Ws[5][0]
    w6p[50:100, 1] = Ws[5][0]
    vecs = np.zeros((HB, 10), f)
    vecs[:, 9] = -2.0
    cx = Ws[0][:, 0]
    ct = Ws[0][:, 1]
    for half in (slice(0, 50), slice(50, 100)):
        vecs[half, 0] = cx
        vecs[half, 1] = ct
        vecs[half, 2] = cx * cx
        vecs[half, 3] = cx * cx * cx
        for l in range(5):
            vecs[half, 4 + l] = bs[l]
    b6bc = np.zeros((128, 2), f)
    b6bc[:, 0] = bs[5][0]
    b6bc[:, 1] = 1.0
    b6bc[B1 - 512:, 1] = 0.0

    maps = []
    for c in range(NCORES):
        sl = x[c * NPC:(c + 1) * NPC]
        h0 = np.zeros((4, FD), f)
        h0[0] = sl[0:FD, 0]
        h0[1] = sl[0:FD, 1]
        h0[2, 0:B1] = sl[FD:NPC, 0]
        h0[3, 0:B1] = sl[FD:NPC, 1]
        paraT = np.ascontiguousarray(para[c * PPC:(c + 1) * PPC].T)
        maps.append({
            "h0": h0, "paraT": paraT, "w1t": w1t, "wb": wb,
            "w6p": w6p, "vecs": vecs, "b6bc": b6bc,
        })
    return maps


_NC_CACHE = {}


def get_program():
    if "nc" not in _NC_CACHE:
        _NC_CACHE["nc"] = build_program()
    return _NC_CACHE["nc"]


def kernel(x, para, W1, b1, W2, b2, W3, b3, W4, b4, W5, b5, W6, b6):
    maps = prep_inputs(x, para, W1, b1, W2, b2, W3, b3, W4, b4, W5, b5, W6, b6)
    nc = get_program()
    res = bass_utils.run_bass_kernel_spmd(nc, maps, list(range(NCORES)))
    out = np.concatenate([res.results[c]["loss"].reshape(-1) for c in range(NCORES)])
    return out.astype(np.float32)



# revision 16
# speedup vs baseline: 1.6158x; 1.6158x over previous
"""Trainium2 Bass kernel for the PINN-style loss problem (v2).

Math: a 6-layer tanh MLP u(x,t) (2->50x5->1) is evaluated with forward-mode
jets (u, u_x, u_t, u_xxx) at N=10000 points. The per-param loss
  loss_p = mean_n (u_t + a_p*u*u_x + b_p*u_xxx + c_p*u_x)^2
collapses to loss_p = ptilde^T G ptilde / N with ptilde = [a,b,c,1] and G the
4x4 Gram of g_n = [u*u_x, u_xxx, u_x, u_t].

v2 design (vs v1):
- No collective. Each core evaluates the tower on its 1250-point x-shard,
  builds its partial Gram G_c, and computes partial losses for ALL 5000
  params q_c[p] = ptilde_p^T G_c ptilde_p / N via one block-diagonal matmul
  against a host-precomputed monomial tensor. The host sums the 8 partial
  loss vectors (loss is linear in G). This removes the AllReduce (9-13us)
  and the old 19us post-AR tail.
- fp16 streams + fp16 matmuls (1 cyc/col at any width; no f32r <256-col
  penalty), DVE 2-byte fast modes for elementwise.
- FD=625 per block (1250 = 2x625): no padded points, no masking.
- Gram via PE transpose of the projected [8,625] stream rows instead of
  20 stationary-stream matmuls.
"""

import os
import sys
import numpy as np

for _p in ("/opt/trn_rl_repo",):
    if os.path.isdir(_p) and _p not in sys.path:
        sys.path.append(_p)

import concourse.bass as bass
import concourse.bacc as bacc
import concourse.mybir as mybir
import concourse.tile as tile
from concourse import bass_utils

F32 = mybir.dt.float32
F16 = mybir.dt.float16
AF = mybir.ActivationFunctionType
ALU = mybir.AluOpType

NCORES = 8
NPTS = 10000
NPC = NPTS // NCORES       # 1250 points per core
FD = 625                   # free dim per block: 2 blocks x 625 = 1250, no pad
HB = 100                   # 2 blocks x 50 hidden units
CHUNKS = ((0, 512), (512, FD - 512))   # matmul free-dim chunks (psum bank)
PG = 8                     # para groups (blockdiag K=128 = 8 groups x 16 rows)
PPG = 5000 // PG           # 625 paras per group
GS = 1e-2                  # Gram-side scale (1/N split as GS*GS_mono)

F32R = mybir.dt.float32r
SDT = F32R if os.environ.get("KSDT") == "f32r" else F16   # stream/weight dtype
NPDT = np.float32 if os.environ.get("KSDT") == "f32r" else np.float16


def _mm(nc, out, lhsT, rhs, start=True, stop=True):
    nc.tensor.matmul(out, lhsT, rhs, start=start, stop=stop)


def _mm_chunks(nc, out_tile, lhsT, rhs_tile, chunks=CHUNKS):
    for off, w in chunks:
        _mm(nc, out_tile[:, off:off + w], lhsT, rhs_tile[:, off:off + w])


def build_program(stage="full"):
    nc = bacc.Bacc("TRN2", target_bir_lowering=False, debug=False)

    h0_d = nc.dram_tensor("h0", [4, FD], SDT, kind="ExternalInput")
    w1t_d = nc.dram_tensor("w1t", [4, HB], SDT, kind="ExternalInput")
    wb_d = nc.dram_tensor("wb", [HB, 400], SDT, kind="ExternalInput")
    w6p_d = nc.dram_tensor("w6p", [HB, 2], SDT, kind="ExternalInput")
    vecs_d = nc.dram_tensor("vecs", [HB, 11], F32, kind="ExternalInput")
    iden_d = nc.dram_tensor("iden8", [8, 8], SDT, kind="ExternalInput")
    mono_d = nc.dram_tensor("mono", [128, PPG], SDT, kind="ExternalInput")
    if stage == "tower":
        loss_d = nc.dram_tensor("dbg", [HB, FD], F32, kind="ExternalOutput")
    elif stage == "gram":
        loss_d = nc.dram_tensor("dbg", [4, 4], F32, kind="ExternalOutput")
    else:
        loss_d = nc.dram_tensor("loss", [PG, PPG], F32, kind="ExternalOutput")

    with tile.TileContext(nc) as tc:
        _body(tc, nc, h0_d, w1t_d, wb_d, w6p_d, vecs_d, iden_d, mono_d,
              loss_d, stage=stage)
    nc.compile()
    return nc


def _body(tc, nc, h0_d, w1t_d, wb_d, w6p_d, vecs_d, iden_d, mono_d, loss_d,
          stage="full"):
    import contextlib

    ctx = contextlib.ExitStack()
    with ctx:
        cpool = ctx.enter_context(tc.tile_pool(name="const", bufs=1))
        spool = ctx.enter_context(tc.tile_pool(name="streams", bufs=2))
        tpool = ctx.enter_context(tc.tile_pool(name="trans", bufs=2))
        dpool = ctx.enter_context(tc.tile_pool(name="dram", bufs=1, space="DRAM"))

        # ---- load constants (split across engine DMA queues) ----
        h0 = cpool.tile([4, FD], SDT, tag="h0")
        w1t = cpool.tile([4, HB], SDT, tag="w1t")
        wb = cpool.tile([HB, 400], SDT, tag="wb")
        w6p = cpool.tile([HB, 2], SDT, tag="w6p")
        vecs = cpool.tile([HB, 11], F32, tag="vecs")
        iden8 = cpool.tile([8, 8], SDT, tag="iden8")
        mono = cpool.tile([128, PPG], SDT, tag="mono")
        gBD = cpool.tile([128, PG], SDT, tag="gBD")

        nc.sync.dma_start(h0[:], h0_d[:])
        nc.sync.dma_start(w1t[:], w1t_d[:])
        nc.scalar.dma_start(vecs[:], vecs_d[:])
        nc.scalar.dma_start(wb[:], wb_d[:])
        nc.gpsimd.dma_start(mono[:], mono_d[:])
        nc.gpsimd.dma_start(w6p[:], w6p_d[:])
        nc.gpsimd.dma_start(iden8[:], iden_d[:])
        nc.gpsimd.memset(gBD[:], 0.0)

        cx = vecs[:, 0:1]
        ct = vecs[:, 1:2]
        cx2 = vecs[:, 2:3]
        cx3 = vecs[:, 3:4]

        def bb(layer):  # bias vector for layer 1..5
            return vecs[:, 3 + layer:4 + layer]

        neg2 = vecs[:, 9:10]
        b6sc = vecs[0:2, 10:11]     # layer-6 bias replicated on 2 partitions

        v = nc.vector
        s = nc.scalar
        g = nc.gpsimd

        with tc.tile_pool(name="ztw", bufs=3, space="PSUM") as zpool:
            # ---------- layer 1 ----------
            z = zpool.tile([HB, FD], F32, tag="ztw")
            _mm_chunks(nc, z, w1t[:], h0)
            a = spool.tile([HB, FD], SDT, tag="a")
            s.activation(a[:], z[:], AF.Tanh, bias=bb(1))
            asq = tpool.tile([HB, FD], SDT, tag="asq")
            s.activation(asq[:], a[:], AF.Square)
            f1 = tpool.tile([HB, FD], SDT, tag="f1")
            v.tensor_scalar(f1[:], asq[:], -1.0, 1.0, ALU.mult, ALU.add)
            h6 = tpool.tile([HB, FD], SDT, tag="h6")
            v.tensor_scalar(h6[:], asq[:], 6.0, -2.0, ALU.mult, ALU.add)
            ax = spool.tile([HB, FD], SDT, tag="ax")
            v.tensor_scalar(ax[:], f1[:], cx, None, ALU.mult)
            at = spool.tile([HB, FD], SDT, tag="at")
            v.tensor_scalar(at[:], f1[:], ct, None, ALU.mult)
            af1 = tpool.tile([HB, FD], SDT, tag="p1")
            g.tensor_tensor(af1[:], a[:], f1[:], ALU.mult)
            axx = spool.tile([HB, FD], SDT, tag="axx")
            v.tensor_scalar(axx[:], af1[:], cx2, -2.0, ALU.mult, ALU.mult)
            f3 = tpool.tile([HB, FD], SDT, tag="p2")
            g.tensor_tensor(f3[:], f1[:], h6[:], ALU.mult)
            axxx = spool.tile([HB, FD], SDT, tag="axxx")
            v.tensor_scalar(axxx[:], f3[:], cx3, None, ALU.mult)

            # ---------- layers 2..5 ----------
            for layer in range(2, 6):
                W = wb[:, 100 * (layer - 2):100 * (layer - 1)]
                last = layer == 5

                z = zpool.tile([HB, FD], F32, tag="ztw")
                _mm_chunks(nc, z, W, a)
                a_n = spool.tile([HB, FD], SDT, tag="a")
                s.activation(a_n[:], z[:], AF.Tanh, bias=bb(layer))

                zx = zpool.tile([HB, FD], F32, tag="ztw")
                _mm_chunks(nc, zx, W, ax)
                # zx copy to SBUF fp16 (consumed 3-4x by DVE fast-mode ops)
                zxC = tpool.tile([HB, FD], SDT, tag="zxC")
                s.activation(zxC[:], zx[:], AF.Copy)
                asq = tpool.tile([HB, FD], SDT, tag="asq")
                v.tensor_tensor(asq[:], a_n[:], a_n[:], ALU.mult)
                f1 = tpool.tile([HB, FD], SDT, tag="f1")
                v.tensor_scalar(f1[:], asq[:], -1.0, 1.0, ALU.mult, ALU.add)
                ax_n = spool.tile([HB, FD], SDT, tag="ax")
                v.tensor_tensor(ax_n[:], f1[:], zxC[:], ALU.mult)
                w2 = tpool.tile([HB, FD], SDT, tag="w2")
                s.activation(w2[:], zxC[:], AF.Square)

                zt = zpool.tile([HB, FD], F32, tag="ztw")
                _mm_chunks(nc, zt, W, at)
                at_n = spool.tile([HB, FD], SDT, tag="at")
                v.tensor_tensor(at_n[:], f1[:], zt[:], ALU.mult)

                zxx = zpool.tile([HB, FD], F32, tag="ztw")
                _mm_chunks(nc, zxx, W, axx)
                zxxC = tpool.tile([HB, FD], SDT, tag="zxxC")
                s.activation(zxxC[:], zxx[:], AF.Copy)
                h6 = tpool.tile([HB, FD], SDT, tag="h6")
                v.tensor_scalar(h6[:], asq[:], 6.0, -2.0, ALU.mult, ALU.add)
                P = tpool.tile([HB, FD], SDT, tag="p1")
                g.tensor_tensor(P[:], a_n[:], zxC[:], ALU.mult)
                zx3 = tpool.tile([HB, FD], SDT, tag="zx3")
                g.tensor_tensor(zx3[:], w2[:], zxC[:], ALU.mult)

                zxxx = zpool.tile([HB, FD], F32, tag="ztw")
                _mm_chunks(nc, zxxx, W, axxx)
                if not last:
                    gt = tpool.tile([HB, FD], SDT, tag="g")
                    g.tensor_tensor(gt[:], a_n[:], w2[:], ALU.mult)
                    inner = tpool.tile([HB, FD], SDT, tag="inner")
                    v.scalar_tensor_tensor(inner[:], gt[:], -2.0, zxxC[:],
                                           ALU.mult, ALU.add)
                m = tpool.tile([HB, FD], SDT, tag="p2")
                v.tensor_tensor(m[:], P[:], zxxC[:], ALU.mult)
                if not last:
                    axx_n = spool.tile([HB, FD], SDT, tag="axx")
                    v.tensor_tensor(axx_n[:], f1[:], inner[:], ALU.mult)

                i3a = tpool.tile([HB, FD], SDT, tag="i3a")
                v.scalar_tensor_tensor(i3a[:], m[:], -6.0, zxxx[:],
                                       ALU.mult, ALU.add)
                n_t = tpool.tile([HB, FD], SDT, tag="n")
                v.tensor_tensor(n_t[:], h6[:], zx3[:], ALU.mult)
                i3 = tpool.tile([HB, FD], SDT, tag="i3")
                v.tensor_tensor(i3[:], i3a[:], n_t[:], ALU.add)
                axxx_n = spool.tile([HB, FD], SDT, tag="axxx")
                v.tensor_tensor(axxx_n[:], f1[:], i3[:], ALU.mult)

                a, at, ax, axxx = a_n, at_n, ax_n, axxx_n
                if not last:
                    axx = axx_n

            a5, ax5, at5, axxx5 = a, ax, at, axxx

        if stage == "tower":
            nc.sync.dma_start(loss_d[:], axxx5[:].bitcast(F32))
            return

        # ---------- layer 6 projection + Gram ----------
        # U8 rows (2s+b): s=0 u, 1 uxxx, 2 ux, 3 ut; b = block.
        with tc.tile_pool(name="proj", bufs=2, space="PSUM") as ppool, \
             tc.tile_pool(name="psmall", bufs=1, space="PSUM") as pps:
            # pair tiles [2,FD] fp16, rows = (block0, block1):
            # puux = (u+b6)*ux, puxxx, pux, put. All partition-0 based.
            # U2 slots rotate (bufs=2); order keeps WAR deps acyclic.
            Ux = ppool.tile([2, FD], F32, tag="U2")
            _mm_chunks(nc, Ux, w6p[:], ax5[:])
            pux = cpool.tile([2, FD], SDT, tag="pux")
            s.activation(pux[:], Ux[:], AF.Copy)
            Uu = ppool.tile([2, FD], F32, tag="U2")
            _mm_chunks(nc, Uu, w6p[:], a5[:])
            puux = cpool.tile([2, FD], SDT, tag="puux")
            v.scalar_tensor_tensor(puux[:], Uu[:], b6sc, pux[:],
                                   ALU.add, ALU.mult)
            Ut = ppool.tile([2, FD], F32, tag="U2")
            _mm_chunks(nc, Ut, w6p[:], at5[:])
            put = cpool.tile([2, FD], SDT, tag="put")
            s.activation(put[:], Ut[:], AF.Copy)
            Uxxx = ppool.tile([2, FD], F32, tag="U2")
            _mm_chunks(nc, Uxxx, w6p[:], axxx5[:])
            puxxx = cpool.tile([2, FD], SDT, tag="puxxx")
            s.activation(puxxx[:], Uxxx[:], AF.Copy)
            pairs = (puux, puxxx, pux, put)

            G4 = pps.tile([4, 4], F32, tag="G4")
            iden2 = iden8[0:2, 0:2]
            TCH = ((0, 128), (128, 128), (256, 128), (384, 128), (512, 113))
            for c, (lo, w) in enumerate(TCH):
                chT_p = pps.tile([128, 8], SDT, tag="chT")
                for sidx, pair in enumerate(pairs):
                    nc.tensor.transpose(chT_p[0:w, 2 * sidx:2 * sidx + 2],
                                        pair[:, lo:lo + w], iden2)
                chT = tpool.tile([128, 8], SDT, tag="chTs")
                v.tensor_copy(chT[0:w, :], chT_p[0:w, :])
                chv = chT[0:w, :].rearrange("p (s b) -> p b s", b=2, s=4)
                for b in range(2):
                    _mm(nc, G4[:], chv[:, b, :], chv[:, b, :],
                        start=(c == 0 and b == 0), stop=(c == 4 and b == 1))

            g16f = cpool.tile([4, 4], SDT, tag="g16f")
            s.activation(g16f[:], G4[:], AF.Copy, scale=GS)

            if stage == "gram":
                gg = cpool.tile([4, 4], F32, tag="gg")
                v.tensor_copy(gg[:], G4[:])
                nc.sync.dma_start(loss_d[:], gg[:])
                return

            # bounce g16 through DRAM into the blockdiag lhsT
            g16d = dpool.tile([1, 16], SDT, tag="g16d")
            nc.sync.dma_start(g16d[:], g16f[:])
            engs = (nc.sync, nc.scalar, nc.gpsimd)
            for bidx in range(PG):
                e = engs[bidx % 3]
                e.dma_start(gBD[16 * bidx:16 * (bidx + 1), bidx:bidx + 1],
                            g16d[:])

            # ---------- partial losses for all 5000 paras ----------
            loss8 = pps.tile([PG, PPG], F32, tag="loss8")
            _mm_chunks(nc, loss8, gBD[:], mono[:])
            lossS = cpool.tile([PG, PPG], F32, tag="lossS")
            s.activation(lossS[:], loss8[:], AF.Copy)
            nc.sync.dma_start(loss_d[:], lossS[:])


def prep_inputs(x, para, W1, b1, W2, b2, W3, b3, W4, b4, W5, b5, W6, b6):
    """Full inputs -> list of per-core input dicts (host-side shard/layout)."""
    f = np.float32
    h = NPDT
    x = np.asarray(x, f)
    para = np.asarray(para, f)
    Ws = [np.asarray(W, f) for W in (W1, W2, W3, W4, W5, W6)]
    bs = [np.asarray(b, f) for b in (b1, b2, b3, b4, b5, b6)]

    w1t = np.zeros((4, HB), h)
    w1t[0:2, 0:50] = Ws[0].T
    w1t[2:4, 50:100] = Ws[0].T
    wb = np.zeros((HB, 400), h)
    for i in range(4):
        W = Ws[i + 1]
        wb[0:50, 100 * i:100 * i + 50] = W.T
        wb[50:100, 100 * i + 50:100 * i + 100] = W.T
    w6p = np.zeros((HB, 2), h)
    w6p[0:50, 0] = Ws[5][0]
    w6p[50:100, 1] = Ws[5][0]
    vecs = np.zeros((HB, 11), f)
    vecs[:, 10] = bs[5][0]
    vecs[:, 9] = -2.0
    cx = Ws[0][:, 0]
    ct = Ws[0][:, 1]
    for half in (slice(0, 50), slice(50, 100)):
        vecs[half, 0] = cx
        vecs[half, 1] = ct
        vecs[half, 2] = cx * cx
        vecs[half, 3] = cx * cx * cx
        for l in range(5):
            vecs[half, 4 + l] = bs[l]
    iden8 = np.eye(8, dtype=h)

    # mono[16*b + 4*i + j, k] = ptilde_i * ptilde_j * GS for para[625*b + k]
    pt = np.concatenate([para, np.ones((5000, 1), f)], axis=1)  # [5000,4]
    mono_full = (pt[:, :, None] * pt[:, None, :] * GS).reshape(5000, 16)
    mono = np.zeros((128, PPG), h)
    for b in range(PG):
        mono[16 * b:16 * (b + 1), :] = mono_full[PPG * b:PPG * (b + 1)].T

    maps = []
    for c in range(NCORES):
        sl = x[c * NPC:(c + 1) * NPC]
        h0 = np.zeros((4, FD), h)
        h0[0] = sl[0:FD, 0]
        h0[1] = sl[0:FD, 1]
        h0[2] = sl[FD:NPC, 0]
        h0[3] = sl[FD:NPC, 1]
        maps.append({
            "h0": h0, "w1t": w1t, "wb": wb, "w6p": w6p, "vecs": vecs,
            "iden8": iden8, "mono": mono,
        })
    return maps


_NC_CACHE = {}


def get_program():
    if "nc" not in _NC_CACHE:
        _NC_CACHE["nc"] = build_program()
    return _NC_CACHE["nc"]


def kernel(x, para, W1, b1, W2, b2, W3, b3, W4, b4, W5, b5, W6, b6):
    maps = prep_inputs(x, para, W1, b1, W2, b2, W3, b3, W4, b4, W5, b5, W6, b6)
    nc = get_program()
    res = bass_utils.run_bass_kernel_spmd(nc, maps, list(range(NCORES)))
    out = np.zeros(5000, np.float64)
    for c in range(NCORES):
        out += res.results[c]["loss"].astype(np.float64).reshape(-1)
    return out.astype(np.float32)


# revision 17
# speedup vs baseline: 1.6218x; 1.0037x over previous
"""Trainium2 Bass kernel for the PINN-style loss problem (v2).

Math: a 6-layer tanh MLP u(x,t) (2->50x5->1) is evaluated with forward-mode
jets (u, u_x, u_t, u_xxx) at N=10000 points. The per-param loss
  loss_p = mean_n (u_t + a_p*u*u_x + b_p*u_xxx + c_p*u_x)^2
collapses to loss_p = ptilde^T G ptilde / N with ptilde = [a,b,c,1] and G the
4x4 Gram of g_n = [u*u_x, u_xxx, u_x, u_t].

v2 design (vs v1):
- No collective. Each core evaluates the tower on its 1250-point x-shard,
  builds its partial Gram G_c, and computes partial losses for ALL 5000
  params q_c[p] = ptilde_p^T G_c ptilde_p / N via one block-diagonal matmul
  against a host-precomputed monomial tensor. The host sums the 8 partial
  loss vectors (loss is linear in G). This removes the AllReduce (9-13us)
  and the old 19us post-AR tail.
- fp16 streams + fp16 matmuls (1 cyc/col at any width; no f32r <256-col
  penalty), DVE 2-byte fast modes for elementwise.
- FD=625 per block (1250 = 2x625): no padded points, no masking.
- Gram via PE transpose of the projected [8,625] stream rows instead of
  20 stationary-stream matmuls.
"""

import os
import sys
import numpy as np

for _p in ("/opt/trn_rl_repo",):
    if os.path.isdir(_p) and _p not in sys.path:
        sys.path.append(_p)

import concourse.bass as bass
import concourse.bacc as bacc
import concourse.mybir as mybir
import concourse.tile as tile
from concourse import bass_utils

F32 = mybir.dt.float32
F16 = mybir.dt.float16
AF = mybir.ActivationFunctionType
ALU = mybir.AluOpType

NCORES = 8
NPTS = 10000
NPC = NPTS // NCORES       # 1250 points per core
FD = 625                   # free dim per block: 2 blocks x 625 = 1250, no pad
HB = 100                   # 2 blocks x 50 hidden units
CHUNKS = ((0, 512), (512, FD - 512))   # matmul free-dim chunks (psum bank)
PG = 8                     # para groups (blockdiag K=128 = 8 groups x 16 rows)
PPG = 5000 // PG           # 625 paras per group
GS = 1e-2                  # Gram-side scale (1/N split as GS*GS_mono)

F32R = mybir.dt.float32r
SDT = F32R if os.environ.get("KSDT") == "f32r" else F16   # stream/weight dtype
NPDT = np.float32 if os.environ.get("KSDT") == "f32r" else np.float16


def _mm(nc, out, lhsT, rhs, start=True, stop=True):
    nc.tensor.matmul(out, lhsT, rhs, start=start, stop=stop)


def _mm_chunks(nc, out_tile, lhsT, rhs_tile, chunks=CHUNKS):
    for off, w in chunks:
        _mm(nc, out_tile[:, off:off + w], lhsT, rhs_tile[:, off:off + w])


def build_program(stage="full"):
    nc = bacc.Bacc("TRN2", target_bir_lowering=False, debug=False)

    h0_d = nc.dram_tensor("h0", [4, FD], SDT, kind="ExternalInput")
    w1t_d = nc.dram_tensor("w1t", [4, HB], SDT, kind="ExternalInput")
    wb_d = nc.dram_tensor("wb", [HB, 400], SDT, kind="ExternalInput")
    w6p_d = nc.dram_tensor("w6p", [HB, 2], SDT, kind="ExternalInput")
    vecs_d = nc.dram_tensor("vecs", [HB, 11], F32, kind="ExternalInput")
    iden_d = nc.dram_tensor("iden8", [8, 8], SDT, kind="ExternalInput")
    mono_d = nc.dram_tensor("mono", [128, PPG], SDT, kind="ExternalInput")
    if stage == "tower":
        loss_d = nc.dram_tensor("dbg", [HB, FD], F32, kind="ExternalOutput")
    elif stage == "gram":
        loss_d = nc.dram_tensor("dbg", [4, 4], F32, kind="ExternalOutput")
    else:
        loss_d = nc.dram_tensor("loss", [PG, PPG], F32, kind="ExternalOutput")

    with tile.TileContext(nc) as tc:
        _body(tc, nc, h0_d, w1t_d, wb_d, w6p_d, vecs_d, iden_d, mono_d,
              loss_d, stage=stage)
    nc.compile()
    return nc


def _body(tc, nc, h0_d, w1t_d, wb_d, w6p_d, vecs_d, iden_d, mono_d, loss_d,
          stage="full"):
    import contextlib

    ctx = contextlib.ExitStack()
    with ctx:
        cpool = ctx.enter_context(tc.tile_pool(name="const", bufs=1))
        spool = ctx.enter_context(tc.tile_pool(name="streams", bufs=2))
        tpool = ctx.enter_context(tc.tile_pool(name="trans", bufs=2))
        dpool = ctx.enter_context(tc.tile_pool(name="dram", bufs=1, space="DRAM"))

        # ---- load constants (split across engine DMA queues) ----
        h0 = cpool.tile([4, FD], SDT, tag="h0")
        w1t = cpool.tile([4, HB], SDT, tag="w1t")
        wb = cpool.tile([HB, 400], SDT, tag="wb")
        w6p = cpool.tile([HB, 2], SDT, tag="w6p")
        vecs = cpool.tile([HB, 11], F32, tag="vecs")
        iden8 = cpool.tile([8, 8], SDT, tag="iden8")
        mono = cpool.tile([128, PPG], SDT, tag="mono")
        gBD = cpool.tile([128, PG], SDT, tag="gBD")

        nc.sync.dma_start(h0[:], h0_d[:])
        nc.sync.dma_start(w1t[:], w1t_d[:])
        nc.scalar.dma_start(vecs[:], vecs_d[:])
        nc.scalar.dma_start(wb[:], wb_d[:])
        nc.gpsimd.dma_start(mono[:], mono_d[:])
        nc.gpsimd.dma_start(w6p[:], w6p_d[:])
        nc.gpsimd.dma_start(iden8[:], iden_d[:])
        nc.gpsimd.memset(gBD[:], 0.0)
        # DVE warmup: the DVE ramps to full clock with a busy streak; burn
        # idle startup time so layer-1 ops run at full speed.
        warm = cpool.tile([HB, FD], SDT, tag="warm")
        nc.vector.memset(warm[:], 1.0)
        for _ in range(4):
            nc.vector.tensor_scalar(warm[:], warm[:], 1.0001, None, ALU.mult)

        cx = vecs[:, 0:1]
        ct = vecs[:, 1:2]
        cx2 = vecs[:, 2:3]
        cx3 = vecs[:, 3:4]

        def bb(layer):  # bias vector for layer 1..5
            return vecs[:, 3 + layer:4 + layer]

        neg2 = vecs[:, 9:10]
        b6sc = vecs[0:2, 10:11]     # layer-6 bias replicated on 2 partitions

        v = nc.vector
        s = nc.scalar
        g = nc.gpsimd

        with tc.tile_pool(name="ztw", bufs=3, space="PSUM") as zpool:
            # ---------- layer 1 ----------
            z = zpool.tile([HB, FD], F32, tag="ztw")
            _mm_chunks(nc, z, w1t[:], h0)
            a = spool.tile([HB, FD], SDT, tag="a")
            s.activation(a[:], z[:], AF.Tanh, bias=bb(1))
            asq = tpool.tile([HB, FD], SDT, tag="asq")
            s.activation(asq[:], a[:], AF.Square)
            f1 = tpool.tile([HB, FD], SDT, tag="f1")
            v.tensor_scalar(f1[:], asq[:], -1.0, 1.0, ALU.mult, ALU.add)
            h6 = tpool.tile([HB, FD], SDT, tag="h6")
            v.tensor_scalar(h6[:], asq[:], 6.0, -2.0, ALU.mult, ALU.add)
            ax = spool.tile([HB, FD], SDT, tag="ax")
            v.tensor_scalar(ax[:], f1[:], cx, None, ALU.mult)
            at = spool.tile([HB, FD], SDT, tag="at")
            v.tensor_scalar(at[:], f1[:], ct, None, ALU.mult)
            af1 = tpool.tile([HB, FD], SDT, tag="p1")
            g.tensor_tensor(af1[:], a[:], f1[:], ALU.mult)
            axx = spool.tile([HB, FD], SDT, tag="axx")
            v.tensor_scalar(axx[:], af1[:], cx2, -2.0, ALU.mult, ALU.mult)
            f3 = tpool.tile([HB, FD], SDT, tag="p2")
            g.tensor_tensor(f3[:], f1[:], h6[:], ALU.mult)
            axxx = spool.tile([HB, FD], SDT, tag="axxx")
            v.tensor_scalar(axxx[:], f3[:], cx3, None, ALU.mult)

            # ---------- layers 2..5 ----------
            for layer in range(2, 6):
                W = wb[:, 100 * (layer - 2):100 * (layer - 1)]
                last = layer == 5

                z = zpool.tile([HB, FD], F32, tag="ztw")
                _mm_chunks(nc, z, W, a)
                a_n = spool.tile([HB, FD], SDT, tag="a")
                s.activation(a_n[:], z[:], AF.Tanh, bias=bb(layer))

                zx = zpool.tile([HB, FD], F32, tag="ztw")
                _mm_chunks(nc, zx, W, ax)
                # zx copy to SBUF fp16 (consumed 3-4x by DVE fast-mode ops)
                zxC = tpool.tile([HB, FD], SDT, tag="zxC")
                s.activation(zxC[:], zx[:], AF.Copy)
                asq = tpool.tile([HB, FD], SDT, tag="asq")
                s.activation(asq[:], a_n[:], AF.Square)
                f1 = tpool.tile([HB, FD], SDT, tag="f1")
                v.tensor_scalar(f1[:], asq[:], -1.0, 1.0, ALU.mult, ALU.add)
                ax_n = spool.tile([HB, FD], SDT, tag="ax")
                v.tensor_tensor(ax_n[:], f1[:], zxC[:], ALU.mult)
                w2 = tpool.tile([HB, FD], SDT, tag="w2")
                s.activation(w2[:], zxC[:], AF.Square)

                zt = zpool.tile([HB, FD], F32, tag="ztw")
                _mm_chunks(nc, zt, W, at)
                at_n = spool.tile([HB, FD], SDT, tag="at")
                v.tensor_tensor(at_n[:], f1[:], zt[:], ALU.mult)

                zxx = zpool.tile([HB, FD], F32, tag="ztw")
                _mm_chunks(nc, zxx, W, axx)
                zxxC = tpool.tile([HB, FD], SDT, tag="zxxC")
                s.activation(zxxC[:], zxx[:], AF.Copy)
                h6 = tpool.tile([HB, FD], SDT, tag="h6")
                v.tensor_scalar(h6[:], asq[:], 6.0, -2.0, ALU.mult, ALU.add)
                P = tpool.tile([HB, FD], SDT, tag="p1")
                g.tensor_tensor(P[:], a_n[:], zxC[:], ALU.mult)
                zx3 = tpool.tile([HB, FD], SDT, tag="zx3")
                g.tensor_tensor(zx3[:], w2[:], zxC[:], ALU.mult)

                zxxx = zpool.tile([HB, FD], F32, tag="ztw")
                _mm_chunks(nc, zxxx, W, axxx)
                if not last:
                    gt = tpool.tile([HB, FD], SDT, tag="g")
                    g.tensor_tensor(gt[:], a_n[:], w2[:], ALU.mult)
                    inner = tpool.tile([HB, FD], SDT, tag="inner")
                    v.scalar_tensor_tensor(inner[:], gt[:], -2.0, zxxC[:],
                                           ALU.mult, ALU.add)
                m = tpool.tile([HB, FD], SDT, tag="p2")
                v.tensor_tensor(m[:], P[:], zxxC[:], ALU.mult)
                if not last:
                    axx_n = spool.tile([HB, FD], SDT, tag="axx")
                    v.tensor_tensor(axx_n[:], f1[:], inner[:], ALU.mult)

                i3a = tpool.tile([HB, FD], SDT, tag="i3a")
                v.scalar_tensor_tensor(i3a[:], m[:], -6.0, zxxx[:],
                                       ALU.mult, ALU.add)
                n_t = tpool.tile([HB, FD], SDT, tag="n")
                v.tensor_tensor(n_t[:], h6[:], zx3[:], ALU.mult)
                i3 = tpool.tile([HB, FD], SDT, tag="i3")
                v.tensor_tensor(i3[:], i3a[:], n_t[:], ALU.add)
                axxx_n = spool.tile([HB, FD], SDT, tag="axxx")
                v.tensor_tensor(axxx_n[:], f1[:], i3[:], ALU.mult)

                a, at, ax, axxx = a_n, at_n, ax_n, axxx_n
                if not last:
                    axx = axx_n

            a5, ax5, at5, axxx5 = a, ax, at, axxx

        if stage == "tower":
            nc.sync.dma_start(loss_d[:], axxx5[:].bitcast(F32))
            return

        # ---------- layer 6 projection + Gram ----------
        # U8 rows (2s+b): s=0 u, 1 uxxx, 2 ux, 3 ut; b = block.
        with tc.tile_pool(name="proj", bufs=2, space="PSUM") as ppool, \
             tc.tile_pool(name="psmall", bufs=1, space="PSUM") as pps:
            # pair tiles [2,FD] fp16, rows = (block0, block1):
            # puux = (u+b6)*ux, puxxx, pux, put. All partition-0 based.
            # U2 slots rotate (bufs=2); order keeps WAR deps acyclic.
            Ux = ppool.tile([2, FD], F32, tag="U2")
            _mm_chunks(nc, Ux, w6p[:], ax5[:])
            pux = cpool.tile([2, FD], SDT, tag="pux")
            s.activation(pux[:], Ux[:], AF.Copy)
            Uu = ppool.tile([2, FD], F32, tag="U2")
            _mm_chunks(nc, Uu, w6p[:], a5[:])
            puux = cpool.tile([2, FD], SDT, tag="puux")
            v.scalar_tensor_tensor(puux[:], Uu[:], b6sc, pux[:],
                                   ALU.add, ALU.mult)
            Ut = ppool.tile([2, FD], F32, tag="U2")
            _mm_chunks(nc, Ut, w6p[:], at5[:])
            put = cpool.tile([2, FD], SDT, tag="put")
            s.activation(put[:], Ut[:], AF.Copy)
            Uxxx = ppool.tile([2, FD], F32, tag="U2")
            _mm_chunks(nc, Uxxx, w6p[:], axxx5[:])
            puxxx = cpool.tile([2, FD], SDT, tag="puxxx")
            s.activation(puxxx[:], Uxxx[:], AF.Copy)
            pairs = (puux, puxxx, pux, put)

            G4 = pps.tile([4, 4], F32, tag="G4")
            iden2 = iden8[0:2, 0:2]
            TCH = ((0, 128), (128, 128), (256, 128), (384, 128), (512, 113))
            for c, (lo, w) in enumerate(TCH):
                chT_p = pps.tile([128, 8], SDT, tag="chT")
                for sidx, pair in enumerate(pairs):
                    nc.tensor.transpose(chT_p[0:w, 2 * sidx:2 * sidx + 2],
                                        pair[:, lo:lo + w], iden2)
                chT = tpool.tile([128, 8], SDT, tag="chTs")
                v.tensor_copy(chT[0:w, :], chT_p[0:w, :])
                chv = chT[0:w, :].rearrange("p (s b) -> p b s", b=2, s=4)
                for b in range(2):
                    _mm(nc, G4[:], chv[:, b, :], chv[:, b, :],
                        start=(c == 0 and b == 0), stop=(c == 4 and b == 1))

            g16f = cpool.tile([4, 4], SDT, tag="g16f")
            s.activation(g16f[:], G4[:], AF.Copy, scale=GS)

            if stage == "gram":
                gg = cpool.tile([4, 4], F32, tag="gg")
                v.tensor_copy(gg[:], G4[:])
                nc.sync.dma_start(loss_d[:], gg[:])
                return

            # bounce g16 through DRAM into the blockdiag lhsT
            g16d = dpool.tile([1, 16], SDT, tag="g16d")
            nc.sync.dma_start(g16d[:], g16f[:])
            engs = (nc.sync, nc.scalar)
            for bidx in range(PG):
                e = engs[bidx % 2]
                e.dma_start(gBD[16 * bidx:16 * (bidx + 1), bidx:bidx + 1],
                            g16d[:])

            # ---------- partial losses for all 5000 paras ----------
            loss8 = pps.tile([PG, PPG], F32, tag="loss8")
            _mm_chunks(nc, loss8, gBD[:], mono[:])
            lossS = cpool.tile([PG, PPG], F32, tag="lossS")
            s.activation(lossS[:], loss8[:], AF.Copy)
            nc.sync.dma_start(loss_d[:], lossS[:])


def prep_inputs(x, para, W1, b1, W2, b2, W3, b3, W4, b4, W5, b5, W6, b6):
    """Full inputs -> list of per-core input dicts (host-side shard/layout)."""
    f = np.float32
    h = NPDT
    x = np.asarray(x, f)
    para = np.asarray(para, f)
    Ws = [np.asarray(W, f) for W in (W1, W2, W3, W4, W5, W6)]
    bs = [np.asarray(b, f) for b in (b1, b2, b3, b4, b5, b6)]

    w1t = np.zeros((4, HB), h)
    w1t[0:2, 0:50] = Ws[0].T
    w1t[2:4, 50:100] = Ws[0].T
    wb = np.zeros((HB, 400), h)
    for i in range(4):
        W = Ws[i + 1]
        wb[0:50, 100 * i:100 * i + 50] = W.T
        wb[50:100, 100 * i + 50:100 * i + 100] = W.T
    w6p = np.zeros((HB, 2), h)
    w6p[0:50, 0] = Ws[5][0]
    w6p[50:100, 1] = Ws[5][0]
    vecs = np.zeros((HB, 11), f)
    vecs[:, 10] = bs[5][0]
    vecs[:, 9] = -2.0
    cx = Ws[0][:, 0]
    ct = Ws[0][:, 1]
    for half in (slice(0, 50), slice(50, 100)):
        vecs[half, 0] = cx
        vecs[half, 1] = ct
        vecs[half, 2] = cx * cx
        vecs[half, 3] = cx * cx * cx
        for l in range(5):
            vecs[half, 4 + l] = bs[l]
    iden8 = np.eye(8, dtype=h)

    # mono[16*b + 4*i + j, k] = ptilde_i * ptilde_j * GS for para[625*b + k]
    pt = np.concatenate([para, np.ones((5000, 1), f)], axis=1)  # [5000,4]
    mono_full = (pt[:, :, None] * pt[:, None, :] * GS).reshape(5000, 16)
    mono = np.zeros((128, PPG), h)
    for b in range(PG):
        mono[16 * b:16 * (b + 1), :] = mono_full[PPG * b:PPG * (b + 1)].T

    maps = []
    for c in range(NCORES):
        sl = x[c * NPC:(c + 1) * NPC]
        h0 = np.zeros((4, FD), h)
        h0[0] = sl[0:FD, 0]
        h0[1] = sl[0:FD, 1]
        h0[2] = sl[FD:NPC, 0]
        h0[3] = sl[FD:NPC, 1]
        maps.append({
            "h0": h0, "w1t": w1t, "wb": wb, "w6p": w6p, "vecs": vecs,
            "iden8": iden8, "mono": mono,
        })
    return maps


_NC_CACHE = {}


def get_program():
    if "nc" not in _NC_CACHE:
        _NC_CACHE["nc"] = build_program()
    return _NC_CACHE["nc"]


def kernel(x, para, W1, b1, W2, b2, W3, b3, W4, b4, W5, b5, W6, b6):
    maps = prep_inputs(x, para, W1, b1, W2, b2, W3, b3, W4, b4, W5, b5, W6, b6)
    nc = get_program()
    res = bass_utils.run_bass_kernel_spmd(nc, maps, list(range(NCORES)))
    out = np.zeros(5000, np.float64)
    for c in range(NCORES):
        out += res.results[c]["loss"].astype(np.float64).reshape(-1)
    return out.astype(np.float32)
